# revision 11
# baseline (speedup 1.0000x reference)
"""LoRA linear on 8 TRN2 NeuronCores.

Computes out = x @ W.T + 2.0 * x @ (lora_B @ lora_A).T for
x:[4,2048,4096] f32, W:[4096,4096] f32, lora_A:[16,4096], lora_B:[4096,16].

Strategy: data-parallel over the 8192 flattened rows of x (1024 rows/core,
params replicated). Per core a single fused kernel:
  stage 1: t.T[r, m]  = sum_k A[r,k] x[m,k]            (rank-16, tiny)
  main:    outT[n, m] = sum_k W[n,k] x[m,k] + sum_r 2*B[n,r] t.T[r,m]
The lora contribution is one extra rank-16 matmul accumulated into the same
PSUM bank as the 32 k-tile main matmuls, so W is streamed exactly once and
x stays resident in SBUF.

All matmul operands are fp16 (fp32 PSUM accumulation); absmax-relative error
vs the fp32 reference is ~3e-4. Host pre-transposes x/W into the layouts the
tensor engine wants (k on partitions) so every DMA is contiguous.
"""

import sys

if "/opt/trn_rl_repo" not in sys.path:
    sys.path.insert(0, "/opt/trn_rl_repo")

import numpy as np

B, S, DIN, DOUT, R = 4, 2048, 4096, 4096, 16
LORA_ALPHA = 2.0
NCORES = 8
M = B * S            # 8192 flattened rows
MSH = M // NCORES    # 1024 rows per core
KT = DIN // 128      # 32 k-tiles
NT = DOUT // 128     # 32 n-tiles
MC = MSH // 512      # 2 moving chunks of 512

TRACE = False        # test.py sets this to get an NTFF profile + exec time
LAST_EXEC_NS = None
LAST_TRACE = None

_compiled = None


def _build():
    import concourse.tile as tile
    import concourse.mybir as mybir
    from concourse import bacc

    DT = mybir.dt.float16
    F32 = mybir.dt.float32

    nc = bacc.Bacc("TRN2", target_bir_lowering=False, debug=False,
                   num_devices=NCORES)
    xT = nc.dram_tensor("xT", [DIN, MSH], DT, kind="ExternalInput").ap()
    w = nc.dram_tensor("w", [NT, 128, DIN], DT, kind="ExternalInput").ap()
    at = nc.dram_tensor("at", [128, KT * R], DT, kind="ExternalInput").ap()
    bt = nc.dram_tensor("bt", [R, DOUT], DT, kind="ExternalInput").ap()
    outT = nc.dram_tensor("outT", [DOUT, MSH], F32, kind="ExternalOutput").ap()

    with tile.TileContext(nc) as tc:
        with tc.tile_pool(name="xt_pool", bufs=1) as xt_pool, \
             tc.tile_pool(name="w_pool", bufs=4) as w_pool, \
             tc.tile_pool(name="small", bufs=1) as small, \
             tc.tile_pool(name="bt_pool", bufs=2) as bt_pool, \
             tc.tile_pool(name="ob_pool", bufs=4) as ob_pool, \
             tc.tile_pool(name="ps_pool", bufs=3, space="PSUM") as ps_pool, \
             tc.tile_pool(name="pt_pool", bufs=2, space="PSUM") as pt_pool:

            # resident x.T, one [128, MSH] tile per k-tile.
            # x/at go on the ACT HWDGE ring, W/bt on the SP ring, output
            # stores on gpsimd SWDGE — three independent DMA paths so the
            # first W tile isn't queued behind 8.4MB of x.
            att = small.tile([128, KT * R], DT, name="att")
            nc.scalar.dma_start(out=att[:], in_=at[:])
            btres = small.tile([R, DOUT], DT, name="btres")
            nc.sync.dma_start(out=btres[:], in_=bt[:])

            xts = []
            for kt in range(KT):
                xt = xt_pool.tile([128, MSH], DT, tag=f"xt{kt}", name=f"xt{kt}")
                nc.scalar.dma_start(out=xt[:], in_=xT[kt * 128:(kt + 1) * 128, :])
                xts.append(xt)

            # stage 1: tT[r, m] = sum_k A[r, k] * x[m, k]
            tT = small.tile([R, MSH], DT, name="tT")
            for mc in range(MC):
                pt = pt_pool.tile([R, 512], F32, name="pt")
                for kt in range(KT):
                    nc.tensor.matmul(
                        pt[:],
                        lhsT=att[:, kt * R:(kt + 1) * R],
                        rhs=xts[kt][:, mc * 512:(mc + 1) * 512],
                        start=(kt == 0), stop=(kt == KT - 1),
                    )
                nc.vector.tensor_copy(tT[:, mc * 512:(mc + 1) * 512], pt[:])

            # main loop over output n-tiles
            for nt in range(NT):
                wt = w_pool.tile([128, DIN], DT, name="wt")
                nc.sync.dma_start(out=wt[:], in_=w[nt])

                pss = [ps_pool.tile([128, 512], F32, name=f"ps{mc}") for mc in range(MC)]
                for kt in range(KT):
                    for mc in range(MC):
                        nc.tensor.matmul(
                            pss[mc][:],
                            lhsT=wt[:, kt * 128:(kt + 1) * 128],
                            rhs=xts[kt][:, mc * 512:(mc + 1) * 512],
                            start=(kt == 0), stop=False,
                        )
                for mc in range(MC):
                    nc.tensor.matmul(
                        pss[mc][:],
                        lhsT=btres[:, nt * 128:(nt + 1) * 128],
                        rhs=tT[:, mc * 512:(mc + 1) * 512],
                        start=False, stop=True,
                    )
                for mc in range(MC):
                    ob = ob_pool.tile([128, 512], F32, name="ob")
                    nc.vector.tensor_copy(ob[:], pss[mc][:])
                    nc.gpsimd.dma_start(
                        out=outT[nt * 128:(nt + 1) * 128, mc * 512:(mc + 1) * 512],
                        in_=ob[:],
                    )

    nc.compile()
    return nc


def kernel(x, weight, lora_A, lora_B):
    global _compiled, LAST_EXEC_NS, LAST_TRACE
    from concourse.bass_utils import run_bass_kernel_spmd

    if _compiled is None:
        _compiled = _build()
    nc = _compiled

    DTnp = np.float16
    xf = np.asarray(x, dtype=np.float32).reshape(M, DIN)
    # w host layout: w[nt, p, kt*128+j] = W[nt*128+j, kt*128+p]
    wt = np.asarray(weight, dtype=np.float32).reshape(NT, 128, KT, 128) \
        .transpose(0, 3, 2, 1).astype(DTnp).reshape(NT, 128, DIN)
    # at[p, kt*16+r] = lora_A[r, kt*128+p]
    at = np.asarray(lora_A, dtype=np.float32).T.reshape(KT, 128, R) \
        .transpose(1, 0, 2).astype(DTnp).reshape(128, KT * R)
    bt = (LORA_ALPHA * np.asarray(lora_B, dtype=np.float32).T).astype(DTnp)
    bt = np.ascontiguousarray(bt)

    in_maps = []
    for i in range(NCORES):
        xTi = np.ascontiguousarray(xf[i * MSH:(i + 1) * MSH].T).astype(DTnp)
        in_maps.append({"xT": xTi, "w": wt, "at": at, "bt": bt})

    trace_cores = [0, 4, 7] if TRACE else None
    res = run_bass_kernel_spmd(nc, in_maps, core_ids=list(range(NCORES)),
                               trace=TRACE, trace_cores=trace_cores)
    LAST_EXEC_NS = res.exec_time_ns
    LAST_TRACE = res.instructions_and_trace

    out = np.empty((M, DOUT), dtype=np.float32)
    for i in range(NCORES):
        out[i * MSH:(i + 1) * MSH] = res.results[i]["outT"].T
    return out.reshape(B, S, DOUT)


# revision 18
# speedup vs baseline: 1.0609x; 1.0609x over previous
"""LoRA linear on 8 TRN2 NeuronCores.

Computes out = x @ W.T + 2.0 * x @ (lora_B @ lora_A).T for
x:[4,2048,4096] f32, W:[4096,4096] f32, lora_A:[16,4096], lora_B:[4096,16].

Strategy: data-parallel over the 8192 flattened rows of x (1024 rows/core,
params replicated). Per core a single fused kernel:
  stage 1: t.T[r, m]  = sum_k A[r,k] x[m,k]            (rank-16, tiny)
  main:    outT[n, m] = sum_k W[n,k] x[m,k] + sum_r 2*B[n,r] t.T[r,m]
The lora contribution is one extra rank-16 matmul accumulated into the same
PSUM bank as the 32 k-tile main matmuls, so W is streamed exactly once and
x stays resident in SBUF.

All matmul operands are fp16 (fp32 PSUM accumulation); absmax-relative error
vs the fp32 reference is ~3e-4. Host pre-transposes x/W into the layouts the
tensor engine wants (k on partitions) so every DMA is contiguous.
"""

import sys

if "/opt/trn_rl_repo" not in sys.path:
    sys.path.insert(0, "/opt/trn_rl_repo")

import numpy as np

B, S, DIN, DOUT, R = 4, 2048, 4096, 4096, 16
LORA_ALPHA = 2.0
NCORES = 8
M = B * S            # 8192 flattened rows
MSH = M // NCORES    # 1024 rows per core
KT = DIN // 128      # 32 k-tiles
NT = DOUT // 128     # 32 n-tiles
MC = MSH // 512      # 2 moving chunks of 512

TRACE = False        # test.py sets this to get an NTFF profile + exec time
LAST_EXEC_NS = None
LAST_TRACE = None

_compiled = None


def _ensure_ntff_hook():
    """This image's antenv package lacks axon_hooks; reconstruct it so the
    trace=True path of run_bass_kernel_spmd can reach the NTFF profiler."""
    try:
        from antenv.axon_hooks import get_axon_ntff_profile_hook  # noqa: F401
        return
    except ImportError:
        pass
    try:
        import types
        import antenv
        from trn_agent_boot.trn_boot import _ntff_profile_via_ctypes

        hook = _ntff_profile_via_ctypes("/opt/axon/libaxon_pjrt.so")
        mod = types.ModuleType("antenv.axon_hooks")
        mod.get_axon_ntff_profile_hook = lambda: hook
        mod.set_axon_ntff_profile_hook = lambda h: None
        sys.modules["antenv.axon_hooks"] = mod
        antenv.axon_hooks = mod
    except Exception:
        pass


def _build():
    import concourse.tile as tile
    import concourse.mybir as mybir
    from concourse import bacc

    DT = mybir.dt.float16
    F32 = mybir.dt.float32

    nc = bacc.Bacc("TRN2", target_bir_lowering=False, debug=False,
                   num_devices=NCORES)
    xT = nc.dram_tensor("xT", [DIN, MSH], DT, kind="ExternalInput").ap()
    w = nc.dram_tensor("w", [NT, 128, DIN], DT, kind="ExternalInput").ap()
    at = nc.dram_tensor("at", [128, KT * R], DT, kind="ExternalInput").ap()
    # bt is 2*lora_B.T zero-padded from 16 to 128 rows: with a full-height
    # stationary, the lora matmul's LDWEIGHTS behaves like every other one
    # (a 16-row load can't use the background weight buffer and stalls the
    # PE ~400ns at every n-tile boundary).
    bt = nc.dram_tensor("bt", [128, DOUT], DT, kind="ExternalInput").ap()
    outT = nc.dram_tensor("outT", [DOUT, MSH], F32, kind="ExternalOutput").ap()

    with tile.TileContext(nc) as tc:
        with tc.tile_pool(name="xt_pool", bufs=1) as xt_pool, \
             tc.tile_pool(name="w_pool", bufs=4) as w_pool, \
             tc.tile_pool(name="small", bufs=1) as small, \
             tc.tile_pool(name="bt_pool", bufs=2) as bt_pool, \
             tc.tile_pool(name="ob_pool", bufs=4) as ob_pool, \
             tc.tile_pool(name="ps_pool", bufs=3, space="PSUM") as ps_pool, \
             tc.tile_pool(name="pt_pool", bufs=2, space="PSUM") as pt_pool:

            # resident x.T, one [128, MSH] tile per k-tile.
            # x/at go on the ACT HWDGE ring, W/bt on the SP ring, output
            # stores on gpsimd SWDGE — three independent DMA paths so the
            # first W tile isn't queued behind 8.4MB of x.
            att = small.tile([128, KT * R], DT, name="att")
            nc.scalar.dma_start(out=att[:], in_=at[:])
            btres = small.tile([128, DOUT], DT, name="btres")
            nc.sync.dma_start(out=btres[:], in_=bt[:])

            xts = []
            for kt in range(KT):
                xt = xt_pool.tile([128, MSH], DT, tag=f"xt{kt}", name=f"xt{kt}")
                nc.scalar.dma_start(out=xt[:], in_=xT[kt * 128:(kt + 1) * 128, :])
                xts.append(xt)

            # stage 1: tT[r, m] = sum_k A[r, k] * x[m, k], rows R..127 zeroed
            # to match the padded bt stationary.
            tT = small.tile([128, MSH], DT, name="tT")
            nc.any.memset(tT[:, :], 0.0)
            for mc in range(MC):
                pt = pt_pool.tile([R, 512], F32, name="pt")
                for kt in range(KT):
                    nc.tensor.matmul(
                        pt[:],
                        lhsT=att[:, kt * R:(kt + 1) * R],
                        rhs=xts[kt][:, mc * 512:(mc + 1) * 512],
                        start=(kt == 0), stop=(kt == KT - 1),
                    )
                nc.vector.tensor_copy(tT[:R, mc * 512:(mc + 1) * 512], pt[:])

            # main loop over output n-tiles
            for nt in range(NT):
                wt = w_pool.tile([128, DIN], DT, name="wt")
                nc.sync.dma_start(out=wt[:], in_=w[nt])

                pss = [ps_pool.tile([128, 512], F32, name=f"ps{mc}") for mc in range(MC)]
                for kt in range(KT):
                    for mc in range(MC):
                        nc.tensor.matmul(
                            pss[mc][:],
                            lhsT=wt[:, kt * 128:(kt + 1) * 128],
                            rhs=xts[kt][:, mc * 512:(mc + 1) * 512],
                            start=(kt == 0), stop=False,
                        )
                for mc in range(MC):
                    nc.tensor.matmul(
                        pss[mc][:],
                        lhsT=btres[:, nt * 128:(nt + 1) * 128],
                        rhs=tT[:, mc * 512:(mc + 1) * 512],
                        start=False, stop=True,
                    )
                for mc in range(MC):
                    ob = ob_pool.tile([128, 512], F32, name="ob")
                    nc.vector.tensor_copy(ob[:], pss[mc][:])
                    nc.gpsimd.dma_start(
                        out=outT[nt * 128:(nt + 1) * 128, mc * 512:(mc + 1) * 512],
                        in_=ob[:],
                    )

    nc.compile()
    return nc


def kernel(x, weight, lora_A, lora_B):
    global _compiled, LAST_EXEC_NS, LAST_TRACE
    from concourse.bass_utils import run_bass_kernel_spmd

    _ensure_ntff_hook()
    if _compiled is None:
        _compiled = _build()
    nc = _compiled

    DTnp = np.float16
    xf = np.asarray(x, dtype=np.float32).reshape(M, DIN)
    # w host layout: w[nt, p, kt*128+j] = W[nt*128+j, kt*128+p]
    wt = np.asarray(weight, dtype=np.float32).reshape(NT, 128, KT, 128) \
        .transpose(0, 3, 2, 1).astype(DTnp).reshape(NT, 128, DIN)
    # at[p, kt*16+r] = lora_A[r, kt*128+p]
    at = np.asarray(lora_A, dtype=np.float32).T.reshape(KT, 128, R) \
        .transpose(1, 0, 2).astype(DTnp).reshape(128, KT * R)
    bt = np.zeros((128, DOUT), dtype=DTnp)
    bt[:R] = (LORA_ALPHA * np.asarray(lora_B, dtype=np.float32).T).astype(DTnp)

    in_maps = []
    for i in range(NCORES):
        xTi = np.ascontiguousarray(xf[i * MSH:(i + 1) * MSH].T).astype(DTnp)
        in_maps.append({"xT": xTi, "w": wt, "at": at, "bt": bt})

    trace_cores = [0, 4, 7] if TRACE else None
    res = run_bass_kernel_spmd(nc, in_maps, core_ids=list(range(NCORES)),
                               trace=TRACE, trace_cores=trace_cores)
    LAST_EXEC_NS = res.exec_time_ns
    LAST_TRACE = res.instructions_and_trace

    out = np.empty((M, DOUT), dtype=np.float32)
    for i in range(NCORES):
        out[i * MSH:(i + 1) * MSH] = res.results[i]["outT"].T
    return out.reshape(B, S, DOUT)


# revision 20
# speedup vs baseline: 1.0717x; 1.0102x over previous
"""LoRA linear on 8 TRN2 NeuronCores.

Computes out = x @ W.T + 2.0 * x @ (lora_B @ lora_A).T for
x:[4,2048,4096] f32, W:[4096,4096] f32, lora_A:[16,4096], lora_B:[4096,16].

Strategy: data-parallel over the 8192 flattened rows of x (1024 rows/core,
params replicated). Per core a single fused kernel:
  stage 1: t.T[r, m]  = sum_k A[r,k] x[m,k]            (rank-16, tiny)
  main:    outT[n, m] = sum_k W[n,k] x[m,k] + sum_r 2*B[n,r] t.T[r,m]
The lora contribution is one extra rank-16 matmul accumulated into the same
PSUM bank as the 32 k-tile main matmuls, so W is streamed exactly once and
x stays resident in SBUF.

All matmul operands are fp16 (fp32 PSUM accumulation); absmax-relative error
vs the fp32 reference is ~3e-4. Host pre-transposes x/W into the layouts the
tensor engine wants (k on partitions) so every DMA is contiguous.
"""

import sys

if "/opt/trn_rl_repo" not in sys.path:
    sys.path.insert(0, "/opt/trn_rl_repo")

import numpy as np

B, S, DIN, DOUT, R = 4, 2048, 4096, 4096, 16
LORA_ALPHA = 2.0
NCORES = 8
M = B * S            # 8192 flattened rows
MSH = M // NCORES    # 1024 rows per core
KT = DIN // 128      # 32 k-tiles
NT = DOUT // 128     # 32 n-tiles
MC = MSH // 512      # 2 moving chunks of 512

TRACE = False        # test.py sets this to get an NTFF profile + exec time
LAST_EXEC_NS = None
LAST_TRACE = None

_compiled = None


def _ensure_ntff_hook():
    """This image's antenv package lacks axon_hooks; reconstruct it so the
    trace=True path of run_bass_kernel_spmd can reach the NTFF profiler."""
    try:
        from antenv.axon_hooks import get_axon_ntff_profile_hook  # noqa: F401
        return
    except ImportError:
        pass
    try:
        import types
        import antenv
        from trn_agent_boot.trn_boot import _ntff_profile_via_ctypes

        hook = _ntff_profile_via_ctypes("/opt/axon/libaxon_pjrt.so")
        mod = types.ModuleType("antenv.axon_hooks")
        mod.get_axon_ntff_profile_hook = lambda: hook
        mod.set_axon_ntff_profile_hook = lambda h: None
        sys.modules["antenv.axon_hooks"] = mod
        antenv.axon_hooks = mod
    except Exception:
        pass


def _build():
    import concourse.tile as tile
    import concourse.mybir as mybir
    from concourse import bacc

    DT = mybir.dt.float16
    F32 = mybir.dt.float32

    nc = bacc.Bacc("TRN2", target_bir_lowering=False, debug=False,
                   num_devices=NCORES)
    xT = nc.dram_tensor("xT", [DIN, MSH], DT, kind="ExternalInput").ap()
    w = nc.dram_tensor("w", [NT, 128, DIN], DT, kind="ExternalInput").ap()
    at = nc.dram_tensor("at", [128, KT * R], DT, kind="ExternalInput").ap()
    # bt is 2*lora_B.T zero-padded from 16 to 128 rows: with a full-height
    # stationary, the lora matmul's LDWEIGHTS behaves like every other one
    # (a 16-row load can't use the background weight buffer and stalls the
    # PE ~400ns at every n-tile boundary).
    bt = nc.dram_tensor("bt", [128, DOUT], DT, kind="ExternalInput").ap()
    outT = nc.dram_tensor("outT", [DOUT, MSH], F32, kind="ExternalOutput").ap()

    with tile.TileContext(nc) as tc:
        with tc.tile_pool(name="xt_pool", bufs=1) as xt_pool, \
             tc.tile_pool(name="w_pool", bufs=4) as w_pool, \
             tc.tile_pool(name="small", bufs=1) as small, \
             tc.tile_pool(name="bt_pool", bufs=2) as bt_pool, \
             tc.tile_pool(name="ob_pool", bufs=4) as ob_pool, \
             tc.tile_pool(name="ps_pool", bufs=3, space="PSUM") as ps_pool, \
             tc.tile_pool(name="pt_pool", bufs=1, space="PSUM") as pt_pool:

            # resident x.T, one [128, MSH] tile per k-tile.
            # x/at go on the ACT HWDGE ring, W/bt on the SP ring, output
            # stores on gpsimd SWDGE — three independent DMA paths so the
            # first W tile isn't queued behind 8.4MB of x.
            att = small.tile([128, KT * R], DT, name="att")
            nc.scalar.dma_start(out=att[:], in_=at[:])
            btres = small.tile([128, DOUT], DT, name="btres")
            nc.sync.dma_start(out=btres[:], in_=bt[:])

            xts = []
            for kt in range(KT):
                xt = xt_pool.tile([128, MSH], DT, tag=f"xt{kt}", name=f"xt{kt}")
                nc.scalar.dma_start(out=xt[:], in_=xT[kt * 128:(kt + 1) * 128, :])
                xts.append(xt)

            # stage 1: tT[r, m] = sum_k A[r, k] * x[m, k], rows R..127 zeroed
            # to match the padded bt stationary. Its matmuls are emitted
            # interleaved with nt=0's k-loop below, so the PE gets 4 matmuls
            # per arriving x-tile instead of 2 while x is still streaming in.
            tT = small.tile([128, MSH], DT, name="tT")
            nc.any.memset(tT[:, :], 0.0)
            pts = [pt_pool.tile([R, 512], F32, name=f"pt{mc}") for mc in range(MC)]

            # main loop over output n-tiles
            for nt in range(NT):
                wt = w_pool.tile([128, DIN], DT, name="wt")
                nc.sync.dma_start(out=wt[:], in_=w[nt])

                pss = [ps_pool.tile([128, 512], F32, name=f"ps{mc}") for mc in range(MC)]
                for kt in range(KT):
                    if nt == 0:
                        for mc in range(MC):
                            nc.tensor.matmul(
                                pts[mc][:],
                                lhsT=att[:, kt * R:(kt + 1) * R],
                                rhs=xts[kt][:, mc * 512:(mc + 1) * 512],
                                start=(kt == 0), stop=(kt == KT - 1),
                            )
                    for mc in range(MC):
                        nc.tensor.matmul(
                            pss[mc][:],
                            lhsT=wt[:, kt * 128:(kt + 1) * 128],
                            rhs=xts[kt][:, mc * 512:(mc + 1) * 512],
                            start=(kt == 0), stop=False,
                        )
                if nt == 0:
                    for mc in range(MC):
                        nc.vector.tensor_copy(
                            tT[:R, mc * 512:(mc + 1) * 512], pts[mc][:])
                for mc in range(MC):
                    nc.tensor.matmul(
                        pss[mc][:],
                        lhsT=btres[:, nt * 128:(nt + 1) * 128],
                        rhs=tT[:, mc * 512:(mc + 1) * 512],
                        start=False, stop=True,
                    )
                for mc in range(MC):
                    ob = ob_pool.tile([128, 512], F32, name="ob")
                    nc.vector.tensor_copy(ob[:], pss[mc][:])
                    nc.gpsimd.dma_start(
                        out=outT[nt * 128:(nt + 1) * 128, mc * 512:(mc + 1) * 512],
                        in_=ob[:],
                    )

    nc.compile()
    return nc


def kernel(x, weight, lora_A, lora_B):
    global _compiled, LAST_EXEC_NS, LAST_TRACE
    from concourse.bass_utils import run_bass_kernel_spmd

    _ensure_ntff_hook()
    if _compiled is None:
        _compiled = _build()
    nc = _compiled

    DTnp = np.float16
    xf = np.asarray(x, dtype=np.float32).reshape(M, DIN)
    # w host layout: w[nt, p, kt*128+j] = W[nt*128+j, kt*128+p]
    wt = np.asarray(weight, dtype=np.float32).reshape(NT, 128, KT, 128) \
        .transpose(0, 3, 2, 1).astype(DTnp).reshape(NT, 128, DIN)
    # at[p, kt*16+r] = lora_A[r, kt*128+p]
    at = np.asarray(lora_A, dtype=np.float32).T.reshape(KT, 128, R) \
        .transpose(1, 0, 2).astype(DTnp).reshape(128, KT * R)
    bt = np.zeros((128, DOUT), dtype=DTnp)
    bt[:R] = (LORA_ALPHA * np.asarray(lora_B, dtype=np.float32).T).astype(DTnp)

    in_maps = []
    for i in range(NCORES):
        xTi = np.ascontiguousarray(xf[i * MSH:(i + 1) * MSH].T).astype(DTnp)
        in_maps.append({"xT": xTi, "w": wt, "at": at, "bt": bt})

    trace_cores = [0, 4, 7] if TRACE else None
    res = run_bass_kernel_spmd(nc, in_maps, core_ids=list(range(NCORES)),
                               trace=TRACE, trace_cores=trace_cores)
    LAST_EXEC_NS = res.exec_time_ns
    LAST_TRACE = res.instructions_and_trace

    out = np.empty((M, DOUT), dtype=np.float32)
    for i in range(NCORES):
        out[i * MSH:(i + 1) * MSH] = res.results[i]["outT"].T
    return out.reshape(B, S, DOUT)


# revision 21
# speedup vs baseline: 1.0750x; 1.0031x over previous
"""LoRA linear on 8 TRN2 NeuronCores.

Computes out = x @ W.T + 2.0 * x @ (lora_B @ lora_A).T for
x:[4,2048,4096] f32, W:[4096,4096] f32, lora_A:[16,4096], lora_B:[4096,16].

Strategy: data-parallel over the 8192 flattened rows of x (1024 rows/core,
params replicated). Per core a single fused kernel:
  stage 1: t.T[r, m]  = sum_k A[r,k] x[m,k]            (rank-16, tiny)
  main:    outT[n, m] = sum_k W[n,k] x[m,k] + sum_r 2*B[n,r] t.T[r,m]
The lora contribution is one extra rank-16 matmul accumulated into the same
PSUM bank as the 32 k-tile main matmuls, so W is streamed exactly once and
x stays resident in SBUF.

All matmul operands are fp16 (fp32 PSUM accumulation); absmax-relative error
vs the fp32 reference is ~3e-4. Host pre-transposes x/W into the layouts the
tensor engine wants (k on partitions) so every DMA is contiguous.
"""

import sys

if "/opt/trn_rl_repo" not in sys.path:
    sys.path.insert(0, "/opt/trn_rl_repo")

import numpy as np

B, S, DIN, DOUT, R = 4, 2048, 4096, 4096, 16
LORA_ALPHA = 2.0
NCORES = 8
M = B * S            # 8192 flattened rows
MSH = M // NCORES    # 1024 rows per core
KT = DIN // 128      # 32 k-tiles
NT = DOUT // 128     # 32 n-tiles
MC = MSH // 512      # 2 moving chunks of 512

TRACE = False        # test.py sets this to get an NTFF profile + exec time
LAST_EXEC_NS = None
LAST_TRACE = None

_compiled = None


def _ensure_ntff_hook():
    """This image's antenv package lacks axon_hooks; reconstruct it so the
    trace=True path of run_bass_kernel_spmd can reach the NTFF profiler."""
    try:
        from antenv.axon_hooks import get_axon_ntff_profile_hook  # noqa: F401
        return
    except ImportError:
        pass
    try:
        import types
        import antenv
        from trn_agent_boot.trn_boot import _ntff_profile_via_ctypes

        hook = _ntff_profile_via_ctypes("/opt/axon/libaxon_pjrt.so")
        mod = types.ModuleType("antenv.axon_hooks")
        mod.get_axon_ntff_profile_hook = lambda: hook
        mod.set_axon_ntff_profile_hook = lambda h: None
        sys.modules["antenv.axon_hooks"] = mod
        antenv.axon_hooks = mod
    except Exception:
        pass


def _build():
    import concourse.tile as tile
    import concourse.mybir as mybir
    from concourse import bacc

    DT = mybir.dt.float16
    F32 = mybir.dt.float32

    nc = bacc.Bacc("TRN2", target_bir_lowering=False, debug=False,
                   num_devices=NCORES)
    xT = nc.dram_tensor("xT", [DIN, MSH], DT, kind="ExternalInput").ap()
    w = nc.dram_tensor("w", [NT, 128, DIN], DT, kind="ExternalInput").ap()
    at = nc.dram_tensor("at", [128, KT * R], DT, kind="ExternalInput").ap()
    # bt is 2*lora_B.T zero-padded from 16 to 128 rows: with a full-height
    # stationary, the lora matmul's LDWEIGHTS behaves like every other one
    # (a 16-row load can't use the background weight buffer and stalls the
    # PE ~400ns at every n-tile boundary).
    bt = nc.dram_tensor("bt", [128, DOUT], DT, kind="ExternalInput").ap()
    outT = nc.dram_tensor("outT", [DOUT, MSH], F32, kind="ExternalOutput").ap()

    with tile.TileContext(nc) as tc:
        with tc.tile_pool(name="xt_pool", bufs=1) as xt_pool, \
             tc.tile_pool(name="w_pool", bufs=4) as w_pool, \
             tc.tile_pool(name="small", bufs=1) as small, \
             tc.tile_pool(name="bt_pool", bufs=2) as bt_pool, \
             tc.tile_pool(name="ob_pool", bufs=4) as ob_pool, \
             tc.tile_pool(name="ps_pool", bufs=3, space="PSUM") as ps_pool, \
             tc.tile_pool(name="pt_pool", bufs=1, space="PSUM") as pt_pool:

            # resident x.T, one [128, MSH] tile per k-tile.
            # x/at go on the ACT HWDGE ring, W/bt on the SP ring, output
            # stores on gpsimd SWDGE — three independent DMA paths so the
            # first W tile isn't queued behind 8.4MB of x.
            att = small.tile([128, KT * R], DT, name="att")
            nc.scalar.dma_start(out=att[:], in_=at[:])
            btres = small.tile([128, DOUT], DT, name="btres")
            nc.sync.dma_start(out=btres[:], in_=bt[:])

            xts = []
            for kt in range(KT):
                xt = xt_pool.tile([128, MSH], DT, tag=f"xt{kt}", name=f"xt{kt}")
                nc.scalar.dma_start(out=xt[:], in_=xT[kt * 128:(kt + 1) * 128, :])
                xts.append(xt)

            # stage 1: tT[r, m] = sum_k A[r, k] * x[m, k], rows R..127 zeroed
            # to match the padded bt stationary. Its matmuls are emitted
            # interleaved with nt=0's k-loop below, so the PE gets 4 matmuls
            # per arriving x-tile instead of 2 while x is still streaming in.
            tT = small.tile([128, MSH], DT, name="tT")
            nc.any.memset(tT[:, :], 0.0)
            pts = [pt_pool.tile([R, 512], F32, name=f"pt{mc}") for mc in range(MC)]

            def emit_main_mms(pss, wt, kt):
                for mc in range(MC):
                    nc.tensor.matmul(
                        pss[mc][:],
                        lhsT=wt[:, kt * 128:(kt + 1) * 128],
                        rhs=xts[kt][:, mc * 512:(mc + 1) * 512],
                        start=(kt == 0), stop=False,
                    )

            def finish_nt(nt, pss):
                for mc in range(MC):
                    nc.tensor.matmul(
                        pss[mc][:],
                        lhsT=btres[:, nt * 128:(nt + 1) * 128],
                        rhs=tT[:, mc * 512:(mc + 1) * 512],
                        start=False, stop=True,
                    )
                for mc in range(MC):
                    ob = ob_pool.tile([128, 512], F32, name="ob")
                    nc.vector.tensor_copy(ob[:], pss[mc][:])
                    nc.gpsimd.dma_start(
                        out=outT[nt * 128:(nt + 1) * 128, mc * 512:(mc + 1) * 512],
                        in_=ob[:],
                    )

            # fused head: stage-1 + nt=0 + nt=1 share one k-loop, so the PE
            # has 6 matmuls per arriving x-tile and stays saturated while x
            # is still streaming from HBM.
            wt0 = w_pool.tile([128, DIN], DT, name="wt", tag="wt")
            nc.sync.dma_start(out=wt0[:], in_=w[0])
            wt1 = w_pool.tile([128, DIN], DT, name="wt", tag="wt")
            nc.sync.dma_start(out=wt1[:], in_=w[1])
            pss0 = [ps_pool.tile([128, 512], F32, name=f"ps{mc}") for mc in range(MC)]
            pss1 = [ps_pool.tile([128, 512], F32, name=f"ps{mc}") for mc in range(MC)]
            for kt in range(KT):
                for mc in range(MC):
                    nc.tensor.matmul(
                        pts[mc][:],
                        lhsT=att[:, kt * R:(kt + 1) * R],
                        rhs=xts[kt][:, mc * 512:(mc + 1) * 512],
                        start=(kt == 0), stop=(kt == KT - 1),
                    )
                emit_main_mms(pss0, wt0, kt)
                emit_main_mms(pss1, wt1, kt)
            for mc in range(MC):
                nc.vector.tensor_copy(tT[:R, mc * 512:(mc + 1) * 512], pts[mc][:])
            finish_nt(0, pss0)
            finish_nt(1, pss1)

            # steady-state loop over the remaining n-tiles
            for nt in range(2, NT):
                wt = w_pool.tile([128, DIN], DT, name="wt", tag="wt")
                nc.sync.dma_start(out=wt[:], in_=w[nt])
                pss = [ps_pool.tile([128, 512], F32, name=f"ps{mc}") for mc in range(MC)]
                for kt in range(KT):
                    emit_main_mms(pss, wt, kt)
                finish_nt(nt, pss)

    nc.compile()
    return nc


def kernel(x, weight, lora_A, lora_B):
    global _compiled, LAST_EXEC_NS, LAST_TRACE
    from concourse.bass_utils import run_bass_kernel_spmd

    _ensure_ntff_hook()
    if _compiled is None:
        _compiled = _build()
    nc = _compiled

    DTnp = np.float16
    xf = np.asarray(x, dtype=np.float32).reshape(M, DIN)
    # w host layout: w[nt, p, kt*128+j] = W[nt*128+j, kt*128+p]
    wt = np.asarray(weight, dtype=np.float32).reshape(NT, 128, KT, 128) \
        .transpose(0, 3, 2, 1).astype(DTnp).reshape(NT, 128, DIN)
    # at[p, kt*16+r] = lora_A[r, kt*128+p]
    at = np.asarray(lora_A, dtype=np.float32).T.reshape(KT, 128, R) \
        .transpose(1, 0, 2).astype(DTnp).reshape(128, KT * R)
    bt = np.zeros((128, DOUT), dtype=DTnp)
    bt[:R] = (LORA_ALPHA * np.asarray(lora_B, dtype=np.float32).T).astype(DTnp)

    in_maps = []
    for i in range(NCORES):
        xTi = np.ascontiguousarray(xf[i * MSH:(i + 1) * MSH].T).astype(DTnp)
        in_maps.append({"xT": xTi, "w": wt, "at": at, "bt": bt})

    trace_cores = [0, 4, 7] if TRACE else None
    res = run_bass_kernel_spmd(nc, in_maps, core_ids=list(range(NCORES)),
                               trace=TRACE, trace_cores=trace_cores)
    LAST_EXEC_NS = res.exec_time_ns
    LAST_TRACE = res.instructions_and_trace

    out = np.empty((M, DOUT), dtype=np.float32)
    for i in range(NCORES):
        out[i * MSH:(i + 1) * MSH] = res.results[i]["outT"].T
    return out.reshape(B, S, DOUT)
